# revision 10
# baseline (speedup 1.0000x reference)
"""GQA attention with QK-norm for Trainium2, sharded over 8 NeuronCores.

Problem: B=2, T=2048, D=2048, H=16 query heads, KVH=4 kv heads, dk=128.
    Q = q @ Wq.T ; K = k @ Wk.T ; V = v @ Wv.T  (per batch)
    Q = g * l2norm(Q, per head) ; K = l2norm(K, per head)
    out = softmax(causal(Q K^T / sqrt(dk))) V @ Wo.T

Sharding: core c = 4*b + gi handles batch b and kv-head group gi
(4 query heads + 1 kv head). Each core computes a row-shard of the
output projection; the host sums the 4 bf16 partials per batch.

fp8 strategy (e4m3):
  - logit path (q, k, Wq*64, Wk*64, normed Q^T*|8g|, K^T*8) is fp8:
    absolute logit noise ~2e-3 is harmless. Q/K projections contract
    D=2048 with DoubleRow perf mode (2 k-tiles per instr = 2x).
  - scores arrive *64; exp runs with scale=1/64 into an f32 stage;
    the Pool engine shifts u = 64*(e^S - 1) into an fp8 strip.
    Deviation encoding keeps fp8 quantization noise ~2e-3 in logits.
  - the "+1" bulk is restored exactly: rowsum = (t+1) + sum(u)/64
    (host iota) and Y += 64*C where C = causal prefix-sums of V,
    computed once per core from (vtm_hi, vtm_lo16) fp8 split via a
    DoubleRow triangular-mask matmul; folded into Y's PSUM with a
    64*I f32r matmul. Y's deviation term u@V_hi only needs V_hi
    (error suppressed by |e^S-1| ~ 0.06).
  - rowsum and Y contract k in DoubleRow pairs (2x); diagonal tiles
    are width-clipped on write, invalid regions zeroed on Pool.
  - V projection and out projection stay bf16 (value path: fp8 would
    inject ~3% output error; 3-chain splits are slower than bf16).
  Validated end-to-end in numpy: rel err 4.6e-3 (gate 2e-2).
"""

import math
import os
import sys

for _p in ("/opt/trn_rl_repo",):
    if _p not in sys.path:
        sys.path.append(_p)

import numpy as np
from concourse import bacc, mybir, tile
from concourse.bass_utils import run_bass_kernel_spmd
from concourse.masks import make_identity

B, T, D, H, KVH, DK = 2, 2048, 2048, 16, 4, 128
HPG = H // KVH          # query heads per core (group)
E = HPG * DK            # 512: q-head dims per core
P = 128
TB = 4                  # t blocks of 512
NT = T // P             # 16 tiles of 128 along T
ND = D // P             # 16 contraction tiles
f32 = mybir.dt.float32
f32r = mybir.dt.float32r
bf16 = mybir.dt.bfloat16
f8 = mybir.dt.float8e4
AF = mybir.ActivationFunctionType
DR = mybir.MatmulPerfMode.DoubleRow
EPS2 = 1e-24


def _f32r(ap):
    return ap.bitcast(f32r)


LVL = int(os.environ.get("ATTN_LVL", "3"))


def build_kernel():
    nc = bacc.Bacc(None, target_bir_lowering=False)

    # host-pre-tiled inputs (see make_in_maps)
    qTt = nc.declare_dram_parameter("qTt", [TB, P, ND, 512], f8, isOutput=False)
    kT = nc.declare_dram_parameter("kT", [ND, P, T], f8, isOutput=False)
    vT = nc.declare_dram_parameter("vT", [ND, P, T], bf16, isOutput=False)
    wqt = nc.declare_dram_parameter("wqt", [P, ND // 2, HPG, 2, DK], f8,
                                    isOutput=False)
    wkt = nc.declare_dram_parameter("wkt", [P, ND // 2, 2, DK], f8,
                                    isOutput=False)
    wvt = nc.declare_dram_parameter("wvt", [P, ND, DK], bf16, isOutput=False)
    wot = nc.declare_dram_parameter("wot", [P, HPG, D], bf16, isOutput=False)
    gs16 = nc.declare_dram_parameter("gs16", [NT, HPG], f32, isOutput=False)
    nrow64 = nc.declare_dram_parameter("nrow64", [P, TB, 4], f32,
                                       isOutput=False)
    outT = nc.declare_dram_parameter("outT", [D, T], bf16, isOutput=True)

    n2_dram = nc.dram_tensor("n2_dram", [HPG + 1, T], f32)
    c_dram = nc.dram_tensor("c_dram", [HPG + 1, T], f32)
    s_dram = nc.dram_tensor("s_dram", [HPG, T], f32)
    inv_dram = nc.dram_tensor("inv_dram", [HPG, T], f32)

    from contextlib import ExitStack

    with tile.TileContext(nc) as tc:
        with ExitStack() as outer:
            const = outer.enter_context(tc.tile_pool(name="const", bufs=1))
            persist = outer.enter_context(tc.tile_pool(name="persist", bufs=1))

            ident = const.tile([P, P], f32, tag="ident")
            make_identity(nc, ident[:])
            ident8p = const.tile([P, 2, P], f8, tag="ident8p")
            nc.vector.tensor_scalar(
                out=ident8p[:, 0, :], in0=ident[:], scalar1=64.0,
                scalar2=None, op0=mybir.AluOpType.mult)
            nc.vector.tensor_scalar(
                out=ident8p[:, 1, :], in0=ident[:], scalar1=4.0,
                scalar2=None, op0=mybir.AluOpType.mult)
            ones_bf = const.tile([P, 1], bf16, tag="ones_bf")
            nc.vector.memset(ones_bf[:], 1.0)
            # DoubleRow ones pair for rowsums
            ones8p = const.tile([P, 2, P], f8, tag="ones8p")
            nc.vector.memset(ones8p[:], 1.0)
            gs_sb = const.tile([NT, HPG], f32, tag="gs")
            nc.sync.dma_start(gs_sb[:], gs16[:])
            nr_sb = const.tile([P, TB, 4], f32, tag="nrow")
            nc.sync.dma_start(nr_sb[:], nrow64[:])
            eps16 = const.tile([NT, 1], f32, tag="eps16")
            nc.vector.memset(eps16[:], EPS2)
            # causal keep-mask: M[p, c] = 1.0 iff c >= p + 384.
            # diagonal k-tile j (0..3) of a 512-wide q block uses
            # M[:, 384-128j : 896-128j]  ==  1{ f >= p + 128 j }.
            maskM = const.tile([P, 896], f32, tag="mask")
            nc.vector.memset(maskM[:], 1.0)
            nc.gpsimd.affine_select(
                out=maskM[:], in_=maskM[:],
                compare_op=mybir.AluOpType.is_ge,
                fill=0.0, base=-384,
                pattern=[[1, 896]], channel_multiplier=-1,
            )
            # tri_pair[:, 0, :] = 1{q >= k} (incl), tri_pair[:, 1, :] same/16
            trif = const.tile([P, P], f32, tag="trif")
            nc.vector.memset(trif[:], 1.0)
            nc.gpsimd.affine_select(
                out=trif[:], in_=trif[:],
                compare_op=mybir.AluOpType.is_ge,
                fill=0.0, base=0,
                pattern=[[1, P]], channel_multiplier=-1,
            )
            tri_pair = const.tile([P, 2, P], f8, tag="tri_pair")
            nc.vector.tensor_copy(tri_pair[:, 0, :], trif[:])
            nc.vector.tensor_scalar(
                out=tri_pair[:, 1, :], in0=trif[:], scalar1=1.0 / 16.0,
                scalar2=None, op0=mybir.AluOpType.mult)

            qt_sb = persist.tile([P, HPG, T], f8, tag="qt")
            kt_sb = persist.tile([P, T], f8, tag="kt")
            vtm_hl = persist.tile([P, NT, 2, P], f8, tag="vtm")
            vtm_y = persist.tile([P, NT, P], f8, tag="vtmy")
            yt_sb = persist.tile([P, HPG, T], bf16, tag="yt")
            c_sb = persist.tile([P, T], f32, tag="csb")
            c8 = persist.tile([P, 2, T], f8, tag="c8")
            prun = persist.tile([P, 1], f32, tag="prun")
            accum_junk = persist.tile([P, 1], f32, tag="ajunk")

            # ---------------- phase A: projections + norms ----------------
            with ExitStack() as pa:
                wpool = pa.enter_context(tc.tile_pool(name="wpool", bufs=1))
                actsq = pa.enter_context(tc.tile_pool(name="actsq", bufs=4))
                actskv = pa.enter_context(tc.tile_pool(name="actskv", bufs=2))
                qkstage = pa.enter_context(tc.tile_pool(name="qkst", bufs=2))
                scratch = pa.enter_context(tc.tile_pool(name="scratch",
                                                        bufs=2))
                bcast = pa.enter_context(tc.tile_pool(name="bcast", bufs=2))
                rows = pa.enter_context(tc.tile_pool(name="rows", bufs=4))
                smal = pa.enter_context(tc.tile_pool(name="smal", bufs=3))
                psA = pa.enter_context(
                    tc.tile_pool(name="psA", bufs=6, space="PSUM"))
                psTP = pa.enter_context(
                    tc.tile_pool(name="psTP", bufs=2, space="PSUM"))

                wk_sb = wpool.tile([P, ND // 2, 2, DK], f8, tag="wk")
                nc.sync.dma_start(wk_sb[:], wkt[:])
                wv_sb = wpool.tile([P, ND, DK], bf16, tag="wv")
                nc.sync.dma_start(wv_sb[:], wvt[:])
                wq_sb = wpool.tile([P, ND // 2, HPG, 2, DK], f8, tag="wq")
                nc.sync.dma_start(wq_sb[:], wqt[:])

                def l2normalize(stage, idx, gs_col, gain_scalar, dst):
                    """Columns of stage [128, T] (bf16) scaled by
                    rsqrt(sum_d x^2) * gain -> dst (fp8)."""
                    sq = scratch.tile([P, T], bf16, tag="scr")
                    nc.vector.tensor_mul(sq[:], stage[:], stage[:])
                    for tb in range(TB):
                        ps = psTP.tile([1, 512], f32, tag="tp")
                        nc.tensor.matmul(
                            ps[:], ones_bf[:],
                            sq[:, tb * 512:(tb + 1) * 512],
                            start=True, stop=True)
                        n2row = rows.tile([1, 512], f32, tag="n2row")
                        nc.vector.tensor_copy(n2row[:], ps[:])
                        nc.sync.dma_start(
                            n2_dram[idx:idx + 1, tb * 512:(tb + 1) * 512],
                            n2row[:])
                    n2c = smal.tile([NT, P], f32, tag="n2c")
                    nc.sync.dma_start(
                        n2c[:], n2_dram[idx, :].rearrange("(c p) -> c p", p=P))
                    # y = rsqrt(n2) with one Newton step
                    sq_c = smal.tile([NT, P], f32, tag="sqc")
                    nc.scalar.activation(sq_c[:], n2c[:], AF.Sqrt,
                                         bias=eps16[:])
                    y0 = smal.tile([NT, P], f32, tag="y0")
                    nc.vector.reciprocal(y0[:], sq_c[:])
                    t1 = smal.tile([NT, P], f32, tag="t1")
                    nc.vector.tensor_mul(t1[:], y0[:], y0[:])
                    nc.vector.tensor_mul(t1[:], t1[:], n2c[:])
                    nc.vector.tensor_scalar(
                        out=t1[:], in0=t1[:], scalar1=-0.5, scalar2=1.5,
                        op0=mybir.AluOpType.mult, op1=mybir.AluOpType.add)
                    nc.vector.tensor_mul(y0[:], y0[:], t1[:])
                    if gs_col is not None:
                        nc.vector.tensor_mul(
                            y0[:], y0[:], gs_col.to_broadcast((NT, P)))
                    if gain_scalar is not None:
                        nc.vector.tensor_scalar(
                            out=y0[:], in0=y0[:], scalar1=gain_scalar,
                            scalar2=None, op0=mybir.AluOpType.mult)
                    nc.sync.dma_start(
                        c_dram[idx, :].rearrange("(c p) -> c p", p=P), y0[:])
                    bc = bcast.tile([P, T], f32, tag="bc")
                    nc.sync.dma_start(
                        bc[:], c_dram[idx:idx + 1, :].to_broadcast((P, T)))
                    nc.vector.tensor_mul(dst, stage[:], bc[:])

                # ---- K projection (fp8 DoubleRow) + norm ----
                kacc = [psA.tile([P, 512], f32, tag="proj",
                                 name=f"kacc{t}") for t in range(TB)]
                for m in range(ND // 2):
                    a = actskv.tile([P, 2, T], f8, tag="akv")
                    nc.sync.dma_start(a[:, 0, :], kT[2 * m])
                    nc.sync.dma_start(a[:, 1, :], kT[2 * m + 1])
                    for tb in range(TB):
                        nc.tensor.matmul(
                            kacc[tb][:],
                            wk_sb[:, m, :, :],
                            a[:, :, tb * 512:(tb + 1) * 512],
                            start=(m == 0), stop=(m == ND // 2 - 1),
                            perf_mode=DR)
                kstage = qkstage.tile([P, T], bf16, tag="kst")
                for tb in range(TB):
                    nc.any.tensor_copy(
                        kstage[:, tb * 512:(tb + 1) * 512], kacc[tb][:])
                l2normalize(kstage, HPG, None, 8.0, kt_sb[:])

                # ---- V projection (bf16) + transpose + fp8 hi/lo split ----
                vacc = [psA.tile([P, 512], f32, tag="proj",
                                 name=f"vacc{t}") for t in range(TB)]
                for n in range(ND):
                    a = actskv.tile([P, 2, T], bf16, tag="akvb")
                    nc.sync.dma_start(a[:, 0, :], vT[n])
                    for tb in range(TB):
                        nc.tensor.matmul(
                            vacc[tb][:],
                            wv_sb[:, n, :],
                            a[:, 0, tb * 512:(tb + 1) * 512],
                            start=(n == 0), stop=(n == ND - 1))
                vt_stage = scratch.tile([P, T], f32, tag="scr")
                for tb in range(TB):
                    nc.any.tensor_copy(
                        vt_stage[:, tb * 512:(tb + 1) * 512], vacc[tb][:])
                nc.vector.memset(prun[:], 0.0)
                for j in range(NT):
                    tp = psTP.tile([P, P], f32, tag="tp")
                    nc.tensor.transpose(
                        tp[:], vt_stage[:, j * P:(j + 1) * P], ident[:])
                    nc.vector.tensor_copy(vtm_hl[:, j, 0, :], tp[:])
                    nc.gpsimd.tensor_copy(vtm_y[:, j, :], vtm_hl[:, j, 0, :])
                    res_bf = smal.tile([P, P], bf16, tag="resbf")
                    nc.vector.tensor_tensor(
                        out=res_bf[:], in0=tp[:], in1=vtm_hl[:, j, 0, :],
                        op=mybir.AluOpType.subtract)
                    nc.vector.tensor_scalar(
                        out=vtm_hl[:, j, 1, :], in0=res_bf[:], scalar1=16.0,
                        scalar2=None, op0=mybir.AluOpType.mult)
                    # C tile: prefix within tile via DR tri matmul
                    cp = psTP.tile([P, P], f32, tag="tp")
                    nc.tensor.matmul(
                        cp[:], vtm_hl[:, j, :, :], tri_pair[:],
                        start=True, stop=True, perf_mode=DR)
                    nc.vector.tensor_scalar(
                        out=c_sb[:, j * P:(j + 1) * P], in0=cp[:],
                        scalar1=prun[:], scalar2=None,
                        op0=mybir.AluOpType.add)
                    nc.vector.tensor_copy(
                        prun[:], c_sb[:, j * P + P - 1:(j + 1) * P])
                    nc.vector.tensor_copy(
                        c8[:, 0, j * P:(j + 1) * P],
                        c_sb[:, j * P:(j + 1) * P])
                    res2_bf = smal.tile([P, P], bf16, tag="resbf")
                    nc.vector.tensor_tensor(
                        out=res2_bf[:], in0=c_sb[:, j * P:(j + 1) * P],
                        in1=c8[:, 0, j * P:(j + 1) * P],
                        op=mybir.AluOpType.subtract)
                    nc.vector.tensor_scalar(
                        out=c8[:, 1, j * P:(j + 1) * P], in0=res2_bf[:],
                        scalar1=16.0, scalar2=None,
                        op0=mybir.AluOpType.mult)

                # ---- Q projection (fp8 DoubleRow) + norms, per head ----
                qa = []
                for tb in range(TB):
                    a = actsq.tile([P, ND, 512], f8, tag="acts",
                                   name=f"qa{tb}")
                    nc.sync.dma_start(a[:], qTt[tb])
                    qa.append(a)
                for h in range(HPG):
                    qstage = qkstage.tile([P, T], bf16, tag="qst")
                    for tb in range(TB):
                        ps = psA.tile([P, 512], f32, tag="proj")
                        for m in range(ND // 2):
                            nc.tensor.matmul(
                                ps[:],
                                wq_sb[:, m, h, :, :],
                                qa[tb][:, 2 * m:2 * m + 2, :],
                                start=(m == 0), stop=(m == ND // 2 - 1),
                                perf_mode=DR)
                        nc.any.tensor_copy(
                            qstage[:, tb * 512:(tb + 1) * 512], ps[:])
                    l2normalize(qstage, h, gs_sb[:, h:h + 1], None,
                                qt_sb[:, h, :])

            # ------------- phase B+C: attention + out projection ----------
            atp = outer.enter_context(tc.tile_pool(name="atp", bufs=2))
            expst = outer.enter_context(tc.tile_pool(name="expst", bufs=3))
            bcy = outer.enter_context(tc.tile_pool(name="bcy", bufs=2))
            invp = outer.enter_context(tc.tile_pool(name="invp", bufs=2))
            wo_pool = outer.enter_context(tc.tile_pool(name="wo", bufs=1))
            ostage = outer.enter_context(tc.tile_pool(name="ostage", bufs=3))
            ps_st = outer.enter_context(
                tc.tile_pool(name="ps_st", bufs=3, space="PSUM"))
            ps_y = outer.enter_context(
                tc.tile_pool(name="ps_y", bufs=2, space="PSUM"))
            ps_sums = outer.enter_context(
                tc.tile_pool(name="ps_sums", bufs=1, space="PSUM"))
            ps_o = outer.enter_context(
                tc.tile_pool(name="ps_o", bufs=2, space="PSUM"))

            wo_sb = wo_pool.tile([P, HPG, D], bf16, tag="wo")
            nc.sync.dma_start(wo_sb[:], wot[:])

            if LVL == 0:
                dump = ostage.tile([P, T], bf16, tag="dump")
                nc.vector.tensor_copy(dump[:], c_sb[:])
                nc.sync.dma_start(outT[0:P, :], dump[:])

            for qb in range(TB if LVL >= 1 else 0):
                n_k = 4 * (qb + 1)
                for h in range(HPG):
                    qh = qt_sb[:, h, qb * 512:(qb + 1) * 512]
                    # stage 1: S -> exp (f32 stage) -> u strip (fp8, Pool)
                    strip = atp.tile([P, NT, 512], f8, tag="strip")
                    for kt in range(n_k):
                        j = kt - 4 * qb
                        off = j * P if j > 0 else 0
                        w = 512 - off
                        st = ps_st.tile([P, 512], f32, tag="st")
                        nc.tensor.matmul(
                            st[:, :w],
                            kt_sb[:, kt * P:(kt + 1) * P],
                            qt_sb[:, h, qb * 512 + off:(qb + 1) * 512],
                            start=True, stop=True)
                        stg = expst.tile([P, 512], f32, tag="expst")
                        nc.scalar.activation(stg[:, :w], st[:, :w], AF.Exp,
                                             scale=1.0 / 64.0)
                        if j > 0:
                            nc.gpsimd.memset(strip[:, kt, :off], 0.0)
                        nc.gpsimd.tensor_scalar(
                            out=strip[:, kt, off:], in0=stg[:, :w],
                            scalar1=1.0, scalar2=64.0,
                            op0=mybir.AluOpType.subtract,
                            op1=mybir.AluOpType.mult)
                        if j >= 0:  # diagonal tile: causal mask
                            nc.gpsimd.tensor_mul(
                                strip[:, kt, off:],
                                strip[:, kt, off:],
                                maskM[:, 384:896 - off])
                    if LVL == 1:
                        dump = ostage.tile([P, 512], bf16, tag="dmp1")
                        nc.vector.tensor_copy(dump[:], strip[:, n_k - 1, :])
                        nc.sync.dma_start(
                            outT[h * P:(h + 1) * P,
                                 qb * 512:(qb + 1) * 512], dump[:])
                        continue
                    # stage 2: rowsums + Y (DoubleRow) + C fold
                    ps_sm = ps_sums.tile([P, 512], f32, tag="sums")
                    for kp in range(n_k // 2):
                        nc.tensor.matmul(
                            ps_sm[:], ones8p[:],
                            strip[:, 2 * kp:2 * kp + 2, :],
                            start=(kp == 0), stop=(kp == n_k // 2 - 1),
                            perf_mode=DR)
                    smrow = invp.tile([1, 512], f32, tag="smrow")
                    nc.any.tensor_copy(smrow[:], ps_sm[0:1, :])
                    nc.sync.dma_start(
                        s_dram[h:h + 1, qb * 512:(qb + 1) * 512], smrow[:])
                    s4 = invp.tile([P, 4], f32, tag="inv")
                    nc.sync.dma_start(
                        s4[:], s_dram[h, qb * 512:(qb + 1) * 512]
                        .rearrange("(p c) -> p c", c=4))
                    nc.vector.tensor_tensor(
                        out=s4[:], in0=s4[:], in1=nr_sb[:, qb, :],
                        op=mybir.AluOpType.add)
                    inv4 = invp.tile([P, 4], f32, tag="inv")
                    nc.vector.reciprocal(inv4[:], s4[:])
                    nc.sync.dma_start(
                        inv_dram[h, qb * 512:(qb + 1) * 512]
                        .rearrange("(p c) -> p c", c=4), inv4[:])

                    ps_yt = ps_y.tile([P, 512], f32, tag="y")
                    for kp in range(n_k // 2):
                        nc.tensor.matmul(
                            ps_yt[:],
                            vtm_y[:, 2 * kp:2 * kp + 2, :],
                            strip[:, 2 * kp:2 * kp + 2, :],
                            start=(kp == 0), stop=False,
                            perf_mode=DR)
                    nc.tensor.matmul(
                        ps_yt[:], ident8p[:],
                        c8[:, :, qb * 512:(qb + 1) * 512],
                        start=False, stop=True, perf_mode=DR)
                    bc = bcy.tile([P, 512], f32, tag="bcy")
                    nc.sync.dma_start(
                        bc[:], inv_dram[h:h + 1, qb * 512:(qb + 1) * 512]
                        .to_broadcast((P, 512)))
                    nc.vector.tensor_mul(
                        yt_sb[:, h, qb * 512:(qb + 1) * 512],
                        ps_yt[:], bc[:])

                # out projection for this t-block (bf16)
                for ot in range(NT if LVL >= 3 else 0):
                    ps = ps_o.tile([P, 512], f32, tag="o")
                    for hh in range(HPG):
                        nc.tensor.matmul(
                            ps[:],
                            wo_sb[:, hh, ot * P:(ot + 1) * P],
                            yt_sb[:, hh, qb * 512:(qb + 1) * 512],
                            start=(hh == 0), stop=(hh == HPG - 1))
                    o_sb = ostage.tile([P, 512], bf16, tag="osb")
                    nc.any.tensor_copy(o_sb[:], ps[:])
                    nc.sync.dma_start(
                        outT[ot * P:(ot + 1) * P, qb * 512:(qb + 1) * 512],
                        o_sb[:])
                if LVL == 2:
                    for hh in range(HPG):
                        dump = ostage.tile([P, 512], bf16, tag="dmp2")
                        nc.vector.tensor_copy(
                            dump[:], yt_sb[:, hh, qb * 512:(qb + 1) * 512])
                        nc.sync.dma_start(
                            outT[hh * P:(hh + 1) * P,
                                 qb * 512:(qb + 1) * 512], dump[:])

    nc.compile()
    return nc


def make_in_maps(q, k, v, Wq, Wk, Wv, Wo, g):
    import ml_dtypes
    s8 = ml_dtypes.float8_e4m3fn
    sb = ml_dtypes.bfloat16
    in_maps = []
    act_t = {}
    for b in range(B):
        qTb = np.ascontiguousarray(q[b].T).astype(s8)
        qTt = np.ascontiguousarray(
            qTb.reshape(ND, P, TB, 512).transpose(2, 1, 0, 3)
            .reshape(TB, P, ND, 512))
        act_t[b] = (
            qTt,
            np.ascontiguousarray(k[b].T).astype(s8).reshape(ND, P, T),
            np.ascontiguousarray(v[b].T).astype(sb).reshape(ND, P, T),
        )

    def wtile(wT, cols, dt, scale=1.0):  # wT: (D, cols) -> [P, D//P, cols]
        return np.ascontiguousarray(
            (np.ascontiguousarray(wT) * scale).reshape(-1, P, cols)
            .transpose(1, 0, 2)).astype(dt)

    g_flat = np.asarray(g, dtype=np.float32).reshape(H)
    tvals = np.arange(1, T + 1, dtype=np.float32) * 64.0
    nrow64 = np.ascontiguousarray(tvals.reshape(TB, P, 4).transpose(1, 0, 2))
    for c in range(8):
        b, gi = divmod(c, KVH)
        qTt, kTb, vTb = act_t[b]
        e0 = gi * E
        # Q gain: S_psum = 64*g/sqrt(dk)*<Qhat,Khat>; K carries x8
        gvals = 8.0 * g_flat[gi * HPG:(gi + 1) * HPG] / math.sqrt(DK)
        in_maps.append({
            "qTt": qTt, "kT": kTb, "vT": vTb,
            "wqt": np.ascontiguousarray(
                wtile(Wq[e0:e0 + E, :].T, E, s8, 64.0)
                .reshape(P, ND // 2, 2, HPG, DK).transpose(0, 1, 3, 2, 4)),
            "wkt": wtile(Wk[gi * DK:(gi + 1) * DK, :].T, DK, s8, 64.0)
                .reshape(P, ND // 2, 2, DK),
            "wvt": wtile(Wv[gi * DK:(gi + 1) * DK, :].T, DK, sb),
            "wot": wtile(Wo[:, e0:e0 + E].T, D, sb).reshape(P, HPG, D),
            "gs16": np.broadcast_to(gvals[None, :], (NT, HPG)).copy(),
            "nrow64": nrow64,
        })
    return in_maps


_cached = {}


def kernel(q, k, v, Wq, Wk, Wv, Wo, g, _trace=False, _tmpdir=None):
    if "nc" not in _cached:
        _cached["nc"] = build_kernel()
    nc = _cached["nc"]
    in_maps = make_in_maps(
        np.asarray(q, np.float32), np.asarray(k, np.float32),
        np.asarray(v, np.float32), np.asarray(Wq, np.float32),
        np.asarray(Wk, np.float32), np.asarray(Wv, np.float32),
        np.asarray(Wo, np.float32), g)
    res = run_bass_kernel_spmd(
        nc, in_maps, list(range(8)), trace=_trace, tmpdir=_tmpdir)
    out = np.empty((B, T, D), dtype=np.float32)
    for b in range(B):
        acc = res.results[4 * b]["outT"].astype(np.float32)
        for gi in range(1, KVH):
            acc += res.results[4 * b + gi]["outT"].astype(np.float32)
        out[b] = acc.T
    kernel.last_results = res
    return out


# revision 14
# speedup vs baseline: 3.8378x; 3.8378x over previous
"""GQA attention with QK-norm for Trainium2, sharded over 8 NeuronCores.

Problem: B=2, T=2048, D=2048, H=16 query heads, KVH=4 kv heads, dk=128.
    Q = q @ Wq.T ; K = k @ Wk.T ; V = v @ Wv.T  (per batch)
    Q = g * l2norm(Q, per head) ; K = l2norm(K, per head)
    out = softmax(causal(Q K^T / sqrt(dk))) V @ Wo.T

Sharding: core c = 4*b + gi handles batch b and kv-head group gi
(4 query heads + 1 kv head). Each core computes a row-shard of the
output projection; the host sums the 4 bf16 partials per batch.

fp8 strategy (e4m3):
  - logit path (q, k, Wq*64, Wk*64, normed Q^T*|8g|, K^T*8) is fp8:
    absolute logit noise ~2e-3 is harmless. Q/K projections contract
    D=2048 with DoubleRow perf mode (2 k-tiles per instr = 2x).
  - scores arrive *64; exp runs with scale=1/64 into an f32 stage;
    the Pool engine shifts u = 64*(e^S - 1) into an fp8 strip.
    Deviation encoding keeps fp8 quantization noise ~2e-3 in logits.
  - the "+1" bulk is restored exactly: rowsum = (t+1) + sum(u)/64
    (host iota) and Y += 64*C where C = causal prefix-sums of V,
    computed once per core from (vtm_hi, vtm_lo16) fp8 split via a
    DoubleRow triangular-mask matmul; folded into Y's PSUM with a
    64*I f32r matmul. Y's deviation term u@V_hi only needs V_hi
    (error suppressed by |e^S-1| ~ 0.06).
  - rowsum and Y contract k in DoubleRow pairs (2x); diagonal tiles
    are width-clipped on write, invalid regions zeroed on Pool.
  - V projection and out projection stay bf16 (value path: fp8 would
    inject ~3% output error; 3-chain splits are slower than bf16).
  Validated end-to-end in numpy: rel err 4.6e-3 (gate 2e-2).
"""

import math
import os
import sys

for _p in ("/opt/trn_rl_repo",):
    if _p not in sys.path:
        sys.path.append(_p)

import numpy as np
from concourse import bacc, mybir, tile
from concourse.bass_utils import run_bass_kernel_spmd
from concourse.masks import make_identity

B, T, D, H, KVH, DK = 2, 2048, 2048, 16, 4, 128
HPG = H // KVH          # query heads per core (group)
E = HPG * DK            # 512: q-head dims per core
P = 128
TB = 4                  # t blocks of 512
NT = T // P             # 16 tiles of 128 along T
ND = D // P             # 16 contraction tiles
f32 = mybir.dt.float32
f32r = mybir.dt.float32r
bf16 = mybir.dt.bfloat16
f8 = mybir.dt.float8e4
AF = mybir.ActivationFunctionType
DR = mybir.MatmulPerfMode.DoubleRow
EPS2 = 1e-24


def _f32r(ap):
    return ap.bitcast(f32r)


LVL = int(os.environ.get("ATTN_LVL", "3"))


def build_kernel():
    nc = bacc.Bacc(None, target_bir_lowering=False)

    # host-pre-tiled inputs (see make_in_maps)
    qTt = nc.declare_dram_parameter("qTt", [TB, P, ND, 512], f8, isOutput=False)
    kT = nc.declare_dram_parameter("kT", [ND, P, T], f8, isOutput=False)
    vT = nc.declare_dram_parameter("vT", [ND, P, T], bf16, isOutput=False)
    wqt = nc.declare_dram_parameter("wqt", [P, ND // 2, HPG, 2, DK], f8,
                                    isOutput=False)
    wkt = nc.declare_dram_parameter("wkt", [P, ND // 2, 2, DK], f8,
                                    isOutput=False)
    wvt = nc.declare_dram_parameter("wvt", [P, ND, DK], bf16, isOutput=False)
    wot = nc.declare_dram_parameter("wot", [P, HPG, D], bf16, isOutput=False)
    gs16 = nc.declare_dram_parameter("gs16", [NT, HPG], f32, isOutput=False)
    nrow64 = nc.declare_dram_parameter("nrow64", [P, TB, 4], f32,
                                       isOutput=False)
    outT = nc.declare_dram_parameter("outT", [D, T], bf16, isOutput=True)

    n2_dram = nc.dram_tensor("n2_dram", [HPG + 1, T], f32)
    c_dram = nc.dram_tensor("c_dram", [HPG + 1, T], f32)
    s_dram = nc.dram_tensor("s_dram", [HPG, T], f32)
    inv_dram = nc.dram_tensor("inv_dram", [HPG, T], f32)

    from contextlib import ExitStack

    with tile.TileContext(nc) as tc:
        with ExitStack() as outer:
            const = outer.enter_context(tc.tile_pool(name="const", bufs=1))
            persist = outer.enter_context(tc.tile_pool(name="persist", bufs=1))

            ident = const.tile([P, P], f32, tag="ident")
            make_identity(nc, ident[:])
            ones_bf = const.tile([P, 1], bf16, tag="ones_bf")
            nc.vector.memset(ones_bf[:], 1.0)
            # DoubleRow ones pair for rowsums
            ones8p = const.tile([P, 2, P], f8, tag="ones8p")
            nc.vector.memset(ones8p[:], 1.0)
            gs_sb = const.tile([NT, HPG], f32, tag="gs")
            nc.sync.dma_start(gs_sb[:], gs16[:])
            nr_sb = const.tile([P, TB, 4], f32, tag="nrow")
            nc.sync.dma_start(nr_sb[:], nrow64[:])
            eps16 = const.tile([NT, 1], f32, tag="eps16")
            nc.vector.memset(eps16[:], EPS2)
            # sliding causal mask source: trimW[p, c] = 64 iff c < p + 384.
            # diag k-tile j uses trimW[:, 384-128j : 896-128j]: nonzero
            # exactly where q_local < k_local + 128 j (the masked region).
            # accumulated into S psum via (-16 I) @ slice = -1024 there.
            trimW = const.tile([P, 896], f32, tag="trimw")
            nc.vector.memset(trimW[:], 64.0)
            nc.gpsimd.affine_select(
                out=trimW[:], in_=trimW[:],
                compare_op=mybir.AluOpType.is_ge,
                fill=0.0, base=383,
                pattern=[[-1, 896]], channel_multiplier=1,
            )
            trimW8 = const.tile([P, 896], f8, tag="trimw8")
            nc.vector.tensor_copy(trimW8[:], trimW[:])
            identm16 = const.tile([P, P], f8, tag="identm16")
            nc.vector.tensor_scalar(
                out=identm16[:], in0=ident[:], scalar1=-16.0, scalar2=None,
                op0=mybir.AluOpType.mult)
            # tri_pair[:, 0, :] = 1{q >= k} (incl), tri_pair[:, 1, :] same/16
            trif = const.tile([P, P], f32, tag="trif")
            nc.vector.memset(trif[:], 1.0)
            nc.gpsimd.affine_select(
                out=trif[:], in_=trif[:],
                compare_op=mybir.AluOpType.is_ge,
                fill=0.0, base=0,
                pattern=[[1, P]], channel_multiplier=-1,
            )
            tri4 = const.tile([P, P], f8, tag="tri4")
            nc.vector.tensor_scalar(
                out=tri4[:], in0=trif[:], scalar1=4.0, scalar2=None,
                op0=mybir.AluOpType.mult)
            ones64c = const.tile([P, 1], f8, tag="ones64c")
            nc.vector.memset(ones64c[:], 64.0)

            qt_sb = persist.tile([P, HPG, T], f8, tag="qt")
            kt_sb = persist.tile([P, T], f8, tag="kt")
            vtm_hl = persist.tile([P, NT, 2, P], f8, tag="vtm")
            vtm_y = persist.tile([P, NT, P], f8, tag="vtmy")
            yt_sb = persist.tile([P, HPG, T], bf16, tag="yt")
            c_sb = persist.tile([P, T], f32, tag="csb")
            p8 = persist.tile([TB, 2, P], f8, tag="p8")
            cbnd = persist.tile([P, TB], f32, tag="cbnd")
            prun = persist.tile([P, 1], f32, tag="prun")
            accum_junk = persist.tile([P, 1], f32, tag="ajunk")

            # ---------------- phase A: projections + norms ----------------
            with ExitStack() as pa:
                wpool = pa.enter_context(tc.tile_pool(name="wpool", bufs=1))
                actsq = pa.enter_context(tc.tile_pool(name="actsq", bufs=4))
                actskv = pa.enter_context(tc.tile_pool(name="actskv", bufs=2))
                qkstage = pa.enter_context(tc.tile_pool(name="qkst", bufs=2))
                scratch = pa.enter_context(tc.tile_pool(name="scratch",
                                                        bufs=2))
                bcast = pa.enter_context(tc.tile_pool(name="bcast", bufs=2))
                rows = pa.enter_context(tc.tile_pool(name="rows", bufs=4))
                smal = pa.enter_context(tc.tile_pool(name="smal", bufs=3))
                psA = pa.enter_context(
                    tc.tile_pool(name="psA", bufs=6, space="PSUM"))
                psTP = pa.enter_context(
                    tc.tile_pool(name="psTP", bufs=2, space="PSUM"))

                wk_sb = wpool.tile([P, ND // 2, 2, DK], f8, tag="wk")
                nc.sync.dma_start(wk_sb[:], wkt[:])
                wv_sb = wpool.tile([P, ND, DK], bf16, tag="wv")
                nc.sync.dma_start(wv_sb[:], wvt[:])
                wq_sb = wpool.tile([P, ND // 2, HPG, 2, DK], f8, tag="wq")
                nc.sync.dma_start(wq_sb[:], wqt[:])

                def l2normalize(stage, idx, gs_col, gain_scalar, dst):
                    """Columns of stage [128, T] (bf16) scaled by
                    rsqrt(sum_d x^2) * gain -> dst (fp8)."""
                    sq = scratch.tile([P, T], bf16, tag="scr")
                    nc.vector.tensor_mul(sq[:], stage[:], stage[:])
                    for tb in range(TB):
                        ps = psTP.tile([1, 512], f32, tag="tp")
                        nc.tensor.matmul(
                            ps[:], ones_bf[:],
                            sq[:, tb * 512:(tb + 1) * 512],
                            start=True, stop=True)
                        n2row = rows.tile([1, 512], f32, tag="n2row")
                        nc.vector.tensor_copy(n2row[:], ps[:])
                        nc.sync.dma_start(
                            n2_dram[idx:idx + 1, tb * 512:(tb + 1) * 512],
                            n2row[:])
                    n2c = smal.tile([NT, P], f32, tag="n2c")
                    nc.sync.dma_start(
                        n2c[:], n2_dram[idx, :].rearrange("(c p) -> c p", p=P))
                    # y = rsqrt(n2) with one Newton step
                    sq_c = smal.tile([NT, P], f32, tag="sqc")
                    nc.scalar.activation(sq_c[:], n2c[:], AF.Sqrt,
                                         bias=eps16[:])
                    y0 = smal.tile([NT, P], f32, tag="y0")
                    nc.vector.reciprocal(y0[:], sq_c[:])
                    t1 = smal.tile([NT, P], f32, tag="t1")
                    nc.vector.tensor_mul(t1[:], y0[:], y0[:])
                    nc.vector.tensor_mul(t1[:], t1[:], n2c[:])
                    nc.vector.tensor_scalar(
                        out=t1[:], in0=t1[:], scalar1=-0.5, scalar2=1.5,
                        op0=mybir.AluOpType.mult, op1=mybir.AluOpType.add)
                    nc.vector.tensor_mul(y0[:], y0[:], t1[:])
                    if gs_col is not None:
                        nc.vector.tensor_mul(
                            y0[:], y0[:], gs_col.to_broadcast((NT, P)))
                    if gain_scalar is not None:
                        nc.vector.tensor_scalar(
                            out=y0[:], in0=y0[:], scalar1=gain_scalar,
                            scalar2=None, op0=mybir.AluOpType.mult)
                    nc.sync.dma_start(
                        c_dram[idx, :].rearrange("(c p) -> c p", p=P), y0[:])
                    bc = bcast.tile([P, T], f32, tag="bc")
                    nc.sync.dma_start(
                        bc[:], c_dram[idx:idx + 1, :].to_broadcast((P, T)))
                    nc.vector.tensor_mul(dst, stage[:], bc[:])

                # ---- K projection (fp8 DoubleRow) + norm ----
                kacc = [psA.tile([P, 512], f32, tag="proj",
                                 name=f"kacc{t}") for t in range(TB)]
                for m in range(ND // 2):
                    a = actskv.tile([P, 2, T], f8, tag="akv")
                    nc.sync.dma_start(a[:, 0, :], kT[2 * m])
                    nc.sync.dma_start(a[:, 1, :], kT[2 * m + 1])
                    for tb in range(TB):
                        nc.tensor.matmul(
                            kacc[tb][:],
                            wk_sb[:, m, :, :],
                            a[:, :, tb * 512:(tb + 1) * 512],
                            start=(m == 0), stop=(m == ND // 2 - 1),
                            perf_mode=DR)
                kstage = qkstage.tile([P, T], bf16, tag="kst")
                for tb in range(TB):
                    nc.any.tensor_copy(
                        kstage[:, tb * 512:(tb + 1) * 512], kacc[tb][:])
                l2normalize(kstage, HPG, None, 8.0, kt_sb[:])

                # ---- V projection (bf16) + transpose + fp8 hi/lo split ----
                vacc = [psA.tile([P, 512], f32, tag="proj",
                                 name=f"vacc{t}") for t in range(TB)]
                for n in range(ND):
                    a = actskv.tile([P, 2, T], bf16, tag="akvb")
                    nc.sync.dma_start(a[:, 0, :], vT[n])
                    for tb in range(TB):
                        nc.tensor.matmul(
                            vacc[tb][:],
                            wv_sb[:, n, :],
                            a[:, 0, tb * 512:(tb + 1) * 512],
                            start=(n == 0), stop=(n == ND - 1))
                vt_stage = scratch.tile([P, T], f32, tag="scr")
                for tb in range(TB):
                    nc.any.tensor_copy(
                        vt_stage[:, tb * 512:(tb + 1) * 512], vacc[tb][:])
                nc.vector.memset(prun[:], 0.0)
                for j in range(NT):
                    tp = psTP.tile([P, P], f32, tag="tp")
                    nc.tensor.transpose(
                        tp[:], vt_stage[:, j * P:(j + 1) * P], ident[:])
                    nc.vector.tensor_copy(vtm_hl[:, j, 0, :], tp[:])
                    nc.gpsimd.tensor_copy(vtm_y[:, j, :], vtm_hl[:, j, 0, :])
                    res_bf = smal.tile([P, P], bf16, tag="resbf")
                    nc.vector.tensor_tensor(
                        out=res_bf[:], in0=tp[:], in1=vtm_hl[:, j, 0, :],
                        op=mybir.AluOpType.subtract)
                    nc.vector.tensor_scalar(
                        out=vtm_hl[:, j, 1, :], in0=res_bf[:], scalar1=16.0,
                        scalar2=None, op0=mybir.AluOpType.mult)
                    # 4*L: within-tile causal prefix of lo16 (x4)
                    cp = psTP.tile([P, P], f32, tag="tp")
                    nc.tensor.matmul(
                        cp[:], vtm_hl[:, j, 1, :], tri4[:],
                        start=True, stop=True)
                    nc.vector.tensor_scalar(
                        out=c_sb[:, j * P:(j + 1) * P], in0=cp[:],
                        scalar1=prun[:], scalar2=None,
                        op0=mybir.AluOpType.add)
                    nc.vector.tensor_copy(
                        prun[:], c_sb[:, j * P + P - 1:(j + 1) * P])


                # 64 * sum_k vtm_hi per tile -> range-prefix boundaries
                hp = psTP.tile([P, NT], f32, tag="tp")
                for j in range(NT):
                    nc.tensor.matmul(
                        hp[:, j:j + 1], vtm_hl[:, j, 0, :], ones64c[:],
                        start=True, stop=True)
                hs = smal.tile([P, NT], f32, tag="hs")
                nc.vector.tensor_copy(hs[:], hp[:])
                for qb4 in range(TB):
                    lo4, hi4 = qb4 * 4, qb4 * 4 + 4
                    nc.vector.tensor_tensor(
                        out=hs[:, lo4 + 1:lo4 + 2], in0=hs[:, lo4:lo4 + 1],
                        in1=hs[:, lo4 + 1:lo4 + 2], op=mybir.AluOpType.add)
                    nc.vector.tensor_tensor(
                        out=hs[:, lo4 + 2:lo4 + 3],
                        in0=hs[:, lo4 + 1:lo4 + 2],
                        in1=hs[:, lo4 + 2:lo4 + 3], op=mybir.AluOpType.add)
                    nc.vector.tensor_tensor(
                        out=hs[:, lo4 + 3:lo4 + 4],
                        in0=hs[:, lo4 + 2:lo4 + 3],
                        in1=hs[:, lo4 + 3:lo4 + 4], op=mybir.AluOpType.add)
                    if qb4 == 0:
                        nc.vector.tensor_copy(
                            cbnd[:, 0:1], hs[:, 3:4])
                    else:
                        nc.vector.tensor_tensor(
                            out=cbnd[:, qb4:qb4 + 1],
                            in0=cbnd[:, qb4 - 1:qb4],
                            in1=hs[:, lo4 + 3:lo4 + 4],
                            op=mybir.AluOpType.add)

                # ---- Q projection (fp8 DoubleRow) + norms, per head ----
                qa = []
                for tb in range(TB):
                    a = actsq.tile([P, ND, 512], f8, tag="acts",
                                   name=f"qa{tb}")
                    nc.sync.dma_start(a[:], qTt[tb])
                    qa.append(a)
                for h in range(HPG):
                    qstage = qkstage.tile([P, T], bf16, tag="qst")
                    for tb in range(TB):
                        ps = psA.tile([P, 512], f32, tag="proj")
                        for m in range(ND // 2):
                            nc.tensor.matmul(
                                ps[:],
                                wq_sb[:, m, h, :, :],
                                qa[tb][:, 2 * m:2 * m + 2, :],
                                start=(m == 0), stop=(m == ND // 2 - 1),
                                perf_mode=DR)
                        nc.any.tensor_copy(
                            qstage[:, tb * 512:(tb + 1) * 512], ps[:])
                    l2normalize(qstage, h, gs_sb[:, h:h + 1], None,
                                qt_sb[:, h, :])

            # ------------- phase B+C: attention + out projection ----------
            atp = outer.enter_context(tc.tile_pool(name="atp", bufs=2))
            expst = outer.enter_context(tc.tile_pool(name="expst", bufs=3))
            bcy = outer.enter_context(tc.tile_pool(name="bcy", bufs=2))
            invp = outer.enter_context(tc.tile_pool(name="invp", bufs=2))
            wo_pool = outer.enter_context(tc.tile_pool(name="wo", bufs=1))
            ostage = outer.enter_context(tc.tile_pool(name="ostage", bufs=3))
            ps_st = outer.enter_context(
                tc.tile_pool(name="ps_st", bufs=2, space="PSUM"))
            ps_y = outer.enter_context(
                tc.tile_pool(name="ps_y", bufs=1, space="PSUM"))
            ps_sums = outer.enter_context(
                tc.tile_pool(name="ps_sums", bufs=1, space="PSUM"))
            ps_o = outer.enter_context(
                tc.tile_pool(name="ps_o", bufs=2, space="PSUM"))

            wo_sb = wo_pool.tile([P, HPG, D], bf16, tag="wo")
            nc.sync.dma_start(wo_sb[:], wot[:])

            if LVL == 0:
                dump = ostage.tile([P, T], bf16, tag="dump")
                nc.vector.tensor_copy(dump[:], c_sb[:])
                nc.sync.dma_start(outT[0:P, :], dump[:])

            for qb in range(TB if LVL >= 1 else 0):
                n_k = 4 * (qb + 1)
                for h in range(HPG):
                    qh = qt_sb[:, h, qb * 512:(qb + 1) * 512]
                    # stage 1: S (+ -1024 on masked diag region)
                    # -> exp pairs -> u = 64 e^S - 64 (masked -> -64)
                    strip = atp.tile([P, NT, 512], f8, tag="strip")
                    for kp in range(n_k // 2):
                        st = ps_st.tile([P, 1024], f32, tag="st")
                        for j2 in range(2):
                            kt = 2 * kp + j2
                            j = kt - 4 * qb
                            sl = st[:, j2 * 512:(j2 + 1) * 512]
                            nc.tensor.matmul(
                                sl,
                                kt_sb[:, kt * P:(kt + 1) * P],
                                qt_sb[:, h, qb * 512:(qb + 1) * 512],
                                start=True, stop=(j < 0))
                            if j >= 0:
                                nc.tensor.matmul(
                                    sl, identm16[:],
                                    trimW8[:, 384 - j * P:896 - j * P],
                                    start=False, stop=True)
                        stg = expst.tile([P, 1024], f32, tag="expst")
                        nc.scalar.activation(stg[:], st[:], AF.Exp,
                                             scale=1.0 / 64.0)
                        uout = strip[:, 2 * kp:2 * kp + 2, :]
                        if kp % 2 == 0:
                            nc.scalar.activation(uout, stg[:], AF.Copy,
                                                 scale=64.0, bias=-64.0)
                        else:
                            nc.vector.tensor_scalar(
                                out=uout, in0=stg[:],
                                scalar1=1.0, scalar2=64.0,
                                op0=mybir.AluOpType.subtract,
                                op1=mybir.AluOpType.mult)
                    # stage 2: rowsums + Y (DoubleRow) + C fold
                    ps_sm = ps_sums.tile([P, 512], f32, tag="sums")
                    for kp in range(n_k // 2):
                        nc.tensor.matmul(
                            ps_sm[:], ones8p[:],
                            strip[:, 2 * kp:2 * kp + 2, :],
                            start=(kp == 0), stop=(kp == n_k // 2 - 1),
                            perf_mode=DR)
                    smrow = invp.tile([1, 512], f32, tag="smrow")
                    nc.any.tensor_copy(smrow[:], ps_sm[0:1, :])
                    nc.sync.dma_start(
                        s_dram[h:h + 1, qb * 512:(qb + 1) * 512], smrow[:])
                    s4 = invp.tile([P, 4], f32, tag="inv")
                    nc.sync.dma_start(
                        s4[:], s_dram[h, qb * 512:(qb + 1) * 512]
                        .rearrange("(p c) -> p c", c=4))
                    nc.vector.tensor_tensor(
                        out=s4[:], in0=s4[:], in1=nr_sb[:, qb, :],
                        op=mybir.AluOpType.add)
                    inv4 = invp.tile([P, 4], f32, tag="inv")
                    nc.vector.reciprocal(inv4[:], s4[:])
                    nc.sync.dma_start(
                        inv_dram[h, qb * 512:(qb + 1) * 512]
                        .rearrange("(p c) -> p c", c=4), inv4[:])

                    ps_yt = ps_y.tile([P, 512], f32, tag="y")
                    for kp in range(n_k // 2):
                        nc.tensor.matmul(
                            ps_yt[:],
                            vtm_y[:, 2 * kp:2 * kp + 2, :],
                            strip[:, 2 * kp:2 * kp + 2, :],
                            start=(kp == 0), stop=(kp == n_k // 2 - 1),
                            perf_mode=DR)

                    bc = bcy.tile([P, 512], f32, tag="bcy")
                    nc.sync.dma_start(
                        bc[:], inv_dram[h:h + 1, qb * 512:(qb + 1) * 512]
                        .to_broadcast((P, 512)))
                    ysum = expst.tile([P, 512], f32, tag="ysum")
                    nc.any.tensor_tensor(
                        out=ysum[:], in0=ps_yt[:],
                        in1=c_sb[:, qb * 512:(qb + 1) * 512],
                        op=mybir.AluOpType.add)
                    nc.vector.tensor_scalar(
                        out=ysum[:], in0=ysum[:],
                        scalar1=cbnd[:, qb:qb + 1], scalar2=None,
                        op0=mybir.AluOpType.add)
                    nc.vector.tensor_mul(
                        yt_sb[:, h, qb * 512:(qb + 1) * 512],
                        ysum[:], bc[:])

                # out projection for this t-block (bf16)
                for ot in range(NT if LVL >= 3 else 0):
                    ps = ps_o.tile([P, 512], f32, tag="o")
                    for hh in range(HPG):
                        nc.tensor.matmul(
                            ps[:],
                            wo_sb[:, hh, ot * P:(ot + 1) * P],
                            yt_sb[:, hh, qb * 512:(qb + 1) * 512],
                            start=(hh == 0), stop=(hh == HPG - 1))
                    o_sb = ostage.tile([P, 512], bf16, tag="osb")
                    nc.any.tensor_copy(o_sb[:], ps[:])
                    nc.sync.dma_start(
                        outT[ot * P:(ot + 1) * P, qb * 512:(qb + 1) * 512],
                        o_sb[:])
                if LVL == 2:
                    for hh in range(HPG):
                        dump = ostage.tile([P, 512], bf16, tag="dmp2")
                        nc.vector.tensor_copy(
                            dump[:], yt_sb[:, hh, qb * 512:(qb + 1) * 512])
                        nc.sync.dma_start(
                            outT[hh * P:(hh + 1) * P,
                                 qb * 512:(qb + 1) * 512], dump[:])

    nc.compile()
    return nc


def make_in_maps(q, k, v, Wq, Wk, Wv, Wo, g):
    import ml_dtypes
    s8 = ml_dtypes.float8_e4m3fn
    sb = ml_dtypes.bfloat16
    in_maps = []
    act_t = {}
    for b in range(B):
        qTb = np.ascontiguousarray(q[b].T).astype(s8)
        qTt = np.ascontiguousarray(
            qTb.reshape(ND, P, TB, 512).transpose(2, 1, 0, 3)
            .reshape(TB, P, ND, 512))
        act_t[b] = (
            qTt,
            np.ascontiguousarray(k[b].T).astype(s8).reshape(ND, P, T),
            np.ascontiguousarray(v[b].T).astype(sb).reshape(ND, P, T),
        )

    def wtile(wT, cols, dt, scale=1.0):  # wT: (D, cols) -> [P, D//P, cols]
        return np.ascontiguousarray(
            (np.ascontiguousarray(wT) * scale).reshape(-1, P, cols)
            .transpose(1, 0, 2)).astype(dt)

    g_flat = np.asarray(g, dtype=np.float32).reshape(H)
    tvals = np.repeat(
        512.0 * np.arange(1, TB + 1, dtype=np.float32), 512) * 64.0
    nrow64 = np.ascontiguousarray(tvals.reshape(TB, P, 4).transpose(1, 0, 2))
    for c in range(8):
        b, gi = divmod(c, KVH)
        qTt, kTb, vTb = act_t[b]
        e0 = gi * E
        # Q gain: S_psum = 64*g/sqrt(dk)*<Qhat,Khat>; K carries x8
        gvals = 8.0 * g_flat[gi * HPG:(gi + 1) * HPG] / math.sqrt(DK)
        in_maps.append({
            "qTt": qTt, "kT": kTb, "vT": vTb,
            "wqt": np.ascontiguousarray(
                wtile(Wq[e0:e0 + E, :].T, E, s8, 64.0)
                .reshape(P, ND // 2, 2, HPG, DK).transpose(0, 1, 3, 2, 4)),
            "wkt": wtile(Wk[gi * DK:(gi + 1) * DK, :].T, DK, s8, 64.0)
                .reshape(P, ND // 2, 2, DK),
            "wvt": wtile(Wv[gi * DK:(gi + 1) * DK, :].T, DK, sb),
            "wot": wtile(Wo[:, e0:e0 + E].T, D, sb).reshape(P, HPG, D),
            "gs16": np.broadcast_to(gvals[None, :], (NT, HPG)).copy(),
            "nrow64": nrow64,
        })
    return in_maps


_cached = {}


def kernel(q, k, v, Wq, Wk, Wv, Wo, g, _trace=False, _tmpdir=None):
    if "nc" not in _cached:
        _cached["nc"] = build_kernel()
    nc = _cached["nc"]
    in_maps = make_in_maps(
        np.asarray(q, np.float32), np.asarray(k, np.float32),
        np.asarray(v, np.float32), np.asarray(Wq, np.float32),
        np.asarray(Wk, np.float32), np.asarray(Wv, np.float32),
        np.asarray(Wo, np.float32), g)
    res = run_bass_kernel_spmd(
        nc, in_maps, list(range(8)), trace=_trace, tmpdir=_tmpdir)
    out = np.empty((B, T, D), dtype=np.float32)
    for b in range(B):
        acc = res.results[4 * b]["outT"].astype(np.float32)
        for gi in range(1, KVH):
            acc += res.results[4 * b + gi]["outT"].astype(np.float32)
        out[b] = acc.T
    kernel.last_results = res
    return out
